# revision 1
# baseline (speedup 1.0000x reference)
"""Trainium2 Bass kernel for nn_MHA_28922309771622.

Multi-head attention with memory prefix (mems prepended to K/V), boolean
mask over KV positions, 16 heads, D=1024, S=2048, MEM=512, fp32.

Sharding: 8 cores = 2 (batch) x 4 (head blocks of 4 heads).  Each core
computes its head block's Q/K/V projections, attention, and the partial
output projection ctx_block @ Wo[:, block].T -> [S, D]; the host sums the
4 head-block partials per batch and adds bo.

Device layout notes (all host-side transposes are free numpy work):
  - Everything is kept "feature on partition" so no on-device transposes
    are needed anywhere.
  - scoresT[kv, s] = K @ Q.T is computed per 128-row kv chunk; exp runs on
    the Scalar engine directly out of PSUM with the mask bias folded into
    the activation's per-partition bias and the 1/sqrt(dh) scale folded
    into the activation scale.  Attention weights are stored bf16.
  - V gets a ones-column appended (at a 128-col padded pitch) so the
    softmax denominator falls out of the context matmul for free.
  - K/V are compacted to unmasked positions only (mask is known at build
    time; masked positions contribute exp(-1e6)=0 exactly), padded to a
    multiple of 128 with -1e6-bias rows.
  - All matmuls run bf16 with fp32 PSUM accumulation and full 128-wide
    operands (zero-padding where needed) so the PE clock gate stays at
    2.4 GHz; per-head softmax denominators are reshaped via DMA so the
    reciprocal runs on all 128 DVE lanes.
"""

import contextlib
import sys

if "/opt/trn_rl_repo" not in sys.path:
    sys.path.insert(0, "/opt/trn_rl_repo")

import ml_dtypes
import numpy as np

import concourse.bass as bass  # noqa: F401
import concourse.mybir as mybir
import concourse.tile as tile
from concourse import bacc
from concourse.bass_utils import run_bass_kernel_spmd

B, S, MEM, D, H = 2, 2048, 512, 1024, 16
DH = D // H            # 64
SKV_FULL = MEM + S     # 2560
N_CORES = 8
HPC = 4                # heads per core
F = HPC * DH           # 256 features per core
NK = D // 128          # 8 contraction chunks over D
NT = F // 128          # 2 feature tiles of 128 per core
FP32 = mybir.dt.float32
FP32R = mybir.dt.float32r
BF16 = mybir.dt.bfloat16
NEG = -1.0e6


def _build(nj: int):
    """Build the SPMD Bass graph for skv_pad = nj*128 kv positions."""
    skv = nj * 128
    nc = bacc.Bacc("TRN2", target_bir_lowering=False, debug=False,
                   num_devices=N_CORES)

    def din(name, shape, dt=FP32):
        return nc.dram_tensor(name, list(shape), dt, kind="ExternalInput").ap()

    xT = din("xT", [D, S], BF16)      # x[b].T
    cT = din("cT", [D, skv], BF16)    # compacted concat(mems,x)[b].T
    wqT = din("wqT", [D, F], BF16)    # Wq[block].T
    wkT = din("wkT", [D, F], BF16)
    wvT = din("wvT", [D, F], BF16)
    woT = din("woT", [F, D], BF16)    # Wo[:, block].T
    mb = din("mb", [128, nj])         # exp bias: 0 kept, -1e6 padding
    ones64_d = din("ones64", [1, 64], BF16)
    bq2 = din("bq2", [128, NT])       # bq[block] feature-major [p, t]
    bk2 = din("bk2", [128, NT])
    bv2 = din("bv2", [128, NT])
    out = nc.dram_tensor("out", [S, D], BF16, kind="ExternalOutput").ap()

    # kv-chunk N-splits for the KT projection (N<=512 per matmul)
    kt_chunks = []
    off = 0
    while off < skv:
        ln = min(512, skv - off)
        kt_chunks.append((off, ln))
        off += ln

    with tile.TileContext(nc) as tc, contextlib.ExitStack() as big:
        pers = big.enter_context(tc.tile_pool(name="pers", bufs=1))
        # xt/wq/wk survive into phase 2: the t=1 halves of the Q/K
        # projections are woven into the first two heads' attention loops
        pql = big.enter_context(tc.tile_pool(name="pql", bufs=1))

        # ---------------- persistent tiles ----------------
        # per-head Q tiles: head h's 64 feature rows sit at the same
        # partition range they occupy in the paired K tile; the other 64
        # rows are zero, so a full K=128 contraction against the paired
        # kt tile yields exactly head h's scores while keeping the PE
        # array fully occupied (the HAM activity monitor throttles the
        # PE clock to 1.2 GHz when the array runs half-empty).
        qt = [pers.tile([128, S], BF16, name=f"qt{h}") for h in range(HPC)]
        kt = [pers.tile([128, skv], BF16, name=f"kt{t}") for t in range(NT)]
        # vaug layout per (j, h): [V_h (64) | ones | zeros (63)] -> full
        # 128-wide stationary operand; the ones column turns the context
        # matmul's row 64 into the softmax denominator.
        vaug = pers.tile([128, nj * (HPC * 128)], BF16, name="vaug")
        wo_sb = [pers.tile([128, D], BF16, name=f"wo{t}") for t in range(NT)]
        mb_sb = pers.tile([128, nj], FP32, name="mb_sb")
        bq_sb = pers.tile([128, NT], FP32, name="bq_sb")
        bk_sb = pers.tile([128, NT], FP32, name="bk_sb")
        bv_sb = pers.tile([128, NT], FP32, name="bv_sb")
        ones64_sb = pers.tile([1, 64], BF16, name="ones64_sb")

        xt_sb = [pql.tile([128, S], BF16, name=f"xt{k}") for k in range(NK)]
        wq_sb = [pql.tile([128, F], BF16, name=f"wq{k}") for k in range(NK)]

        vview = vaug.rearrange("p (j h e) -> p j h e", j=nj, h=HPC, e=128)

        def evict_q(ps, t, nsl):
            for r in range(2):
                rsl = slice(r * 64, (r + 1) * 64)
                nc.vector.tensor_scalar_add(
                    qt[2 * t + r][rsl, nsl], ps[rsl, :], bq_sb[rsl, t:t + 1])

        # ============ phase 1: K/V + Q(t=0) projections ============
        with contextlib.ExitStack() as st1:
            proj = st1.enter_context(tc.tile_pool(name="proj", bufs=1))
            psA = st1.enter_context(
                tc.tile_pool(name="psA", bufs=2, space="PSUM"))

            ct_sb = [proj.tile([128, skv], BF16, name=f"ct{k}")
                     for k in range(NK)]
            wv_sb = [proj.tile([128, F], BF16, name=f"wv{k}")
                     for k in range(NK)]
            wk_sb = [proj.tile([128, F], BF16, name=f"wk{k}")
                     for k in range(NK)]
            wz = proj.tile([128, 512], BF16, name="wz")

            # PE warmup: zero-dep matmuls keep the Tensor engine busy
            # while the input DMAs land, so the HAM clock gate grants the
            # 2.4 GHz clock before the real projections begin.  The wz
            # memset must be the first DVE op or the warmup waits behind
            # the big vaug/qt memsets.
            nc.vector.memset(wz[:], 1.0)
            psw = psA.tile([128, 512], FP32, tag="psA", name="psw")
            for i in range(56):
                nc.tensor.matmul(psw[:], wz[:, :128], wz[:],
                                 start=(i == 0), stop=(i == 55))

            nc.vector.memset(vview[:, :, :, 64:65], 1.0)
            nc.vector.memset(vview[:, :, :, 65:128], 0.0)
            for h in range(HPC):
                zsl = slice(64, 128) if h % 2 == 0 else slice(0, 64)
                nc.vector.memset(qt[h][zsl, :], 0.0)

            # small tiles first so nothing downstream waits on the bulk
            nc.sync.dma_start(mb_sb[:], mb[:])
            nc.sync.dma_start(bq_sb[:], bq2[:])
            nc.sync.dma_start(bk_sb[:], bk2[:])
            nc.sync.dma_start(bv_sb[:], bv2[:])
            nc.sync.dma_start(ones64_sb[:], ones64_d[:])
            for k in range(NK):
                nc.sync.dma_start(wk_sb[k][:], wkT[k * 128:(k + 1) * 128, :])
            for k in range(NK):
                nc.sync.dma_start(ct_sb[k][:], cT[k * 128:(k + 1) * 128, :])
            for k in range(NK):
                nc.sync.dma_start(wv_sb[k][:], wvT[k * 128:(k + 1) * 128, :])
            for k in range(NK):
                nc.sync.dma_start(wq_sb[k][:], wqT[k * 128:(k + 1) * 128, :])
            for half in range(2):
                hsl = slice(half * 1024, (half + 1) * 1024)
                for k in range(NK):
                    nc.sync.dma_start(xt_sb[k][:, hsl],
                                      xT[k * 128:(k + 1) * 128, hsl])
            for t in range(NT):
                nc.sync.dma_start(wo_sb[t][:], woT[t * 128:(t + 1) * 128, :])

            # KT (both halves)
            for t in range(NT):
                fsl = slice(t * 128, (t + 1) * 128)
                for (noff, nlen) in kt_chunks:
                    ps = psA.tile([128, 512], FP32, tag="psA", name="ps_k")
                    for k in range(NK):
                        nc.tensor.matmul(
                            ps[:, :nlen], wk_sb[k][:, fsl],
                            ct_sb[k][:, noff:noff + nlen],
                            start=(k == 0), stop=(k == NK - 1))
                    nc.vector.tensor_scalar_add(
                        kt[t][:, noff:noff + nlen], ps[:, :nlen],
                        bk_sb[:, t:t + 1])

            # V (fills the xt-DMA wait)
            for j in range(nj):
                ps = psA.tile([128, 512], FP32, tag="psA", name="ps_v")
                for k in range(NK):
                    nc.tensor.matmul(
                        ps[:, :F], ct_sb[k][:, j * 128:(j + 1) * 128],
                        wv_sb[k][:], start=(k == 0), stop=(k == NK - 1))
                pv = ps[:, :F].rearrange("p (h e) -> p h e", h=HPC)
                nc.vector.tensor_copy(vview[:, j, :, 0:64], pv)

            # QT t=0 (heads 0, 1)
            for n in range(S // 512):
                ps = psA.tile([128, 512], FP32, tag="psA", name="ps_q")
                for k in range(NK):
                    nc.tensor.matmul(
                        ps[:], wq_sb[k][:, 0:128],
                        xt_sb[k][:, n * 512:(n + 1) * 512],
                        start=(k == 0), stop=(k == NK - 1))
                evict_q(ps, 0, slice(n * 512, (n + 1) * 512))

        # ============ phase 2: attention (+ Q/K t=1 woven in) ============
        with contextlib.ExitStack() as st2:
            late = st2.enter_context(tc.tile_pool(name="late", bufs=1))
            epool = st2.enter_context(tc.tile_pool(name="epool", bufs=13))
            psA2 = st2.enter_context(
                tc.tile_pool(name="psA2", bufs=2, space="PSUM"))
            psC2 = st2.enter_context(
                tc.tile_pool(name="psC2", bufs=2, space="PSUM"))

            ctxb = [late.tile([128, S], BF16, name=f"ctxb{t}")
                    for t in range(NT)]
            # denominators: packed [128, 16] per head so the DVE
            # reciprocal runs on all 128 lanes (a [1, S] reciprocal is
            # ~13us on HW; this is <1us)
            dpack = late.tile([128, HPC * 16], FP32, name="dpack")
            dtmp = [late.tile([1, S], FP32, name=f"dtmp{h}", tag="dtmp")
                    for h in range(HPC)]
            rpack = late.tile([128, HPC * 16], BF16, name="rpack")
            recip = [late.tile([1, S], BF16, name=f"recip{h}")
                     for h in range(HPC)]

            etiles = {}     # (h, j) -> expT tile [128, S] bf16
            ctx_ps = {}     # h -> psum accumulator [128, S] (rows 0-64 live)

            def emit_scores(h, j):
                t = h // 2
                e = epool.tile([128, S], BF16, tag="expT", name=f"e{h}_{j}")
                etiles[(h, j)] = e
                for half in range(2):
                    ps = psA2.tile([128, 1024], FP32, tag="psS", name="ps_s")
                    for n in range(2):
                        ssl = slice((half * 2 + n) * 512,
                                    (half * 2 + n + 1) * 512)
                        nc.tensor.matmul(
                            ps[:, n * 512:(n + 1) * 512],
                            kt[t][:, j * 128:(j + 1) * 128],
                            qt[h][:, ssl],
                            start=True, stop=True)
                    nc.scalar.activation(
                        e[:, half * 1024:(half + 1) * 1024], ps[:],
                        mybir.ActivationFunctionType.Exp,
                        bias=mb_sb[:, j:j + 1], scale=1.0 / 8.0)

            def emit_qt1_group(n):
                nsl = slice(n * 256, (n + 1) * 256)
                ps = psA2.tile([128, 256], FP32, tag="psS", name="ps_q1")
                for k in range(NK):
                    nc.tensor.matmul(
                        ps[:], wq_sb[k][:, 128:256], xt_sb[k][:, nsl],
                        start=(k == 0), stop=(k == NK - 1))
                evict_q(ps, 1, nsl)

            def emit_ctx_step(h, j):
                if j == 0:
                    ctx_ps[h] = [psC2.tile([128, 1024], FP32, tag="psC",
                                           name=f"ctx{h}_{sh}")
                                 for sh in range(2)]
                pca, pcb = ctx_ps[h]
                e = etiles[(h, j)]
                vsl = vaug[:, j * (HPC * 128) + h * 128:
                           j * (HPC * 128) + (h + 1) * 128]
                for n in range(4):
                    ssl = slice(n * 512, (n + 1) * 512)
                    pc = pca if n < 2 else pcb
                    nc.tensor.matmul(
                        pc[:, (n % 2) * 512:(n % 2 + 1) * 512],
                        vsl, e[:, ssl],
                        start=(j == 0), stop=(j == nj - 1))

            def emit_evict(h):
                t, r = divmod(h, 2)
                rsl = slice(r * 64, (r + 1) * 64)
                pca, pcb = ctx_ps.pop(h)
                nc.vector.tensor_copy(dtmp[h][:, 0:1024], pca[64:65, :])
                nc.vector.tensor_copy(dtmp[h][:, 1024:2048], pcb[64:65, :])
                nc.vector.tensor_copy(ctxb[t][rsl, 0:1024], pca[0:64, :])
                nc.vector.tensor_copy(ctxb[t][rsl, 1024:2048], pcb[0:64, :])
                # reshape 1x2048 -> 128x16 via DMA so the reciprocal runs
                # on all lanes, then spread it back
                nc.sync.dma_start(dpack[:, h * 16:(h + 1) * 16], dtmp[h][:])
                with nc.allow_low_precision(
                        reason="bf16 recip feeds bf16 broadcast mm"):
                    nc.vector.reciprocal(rpack[:, h * 16:(h + 1) * 16],
                                         dpack[:, h * 16:(h + 1) * 16])
                nc.sync.dma_start(recip[h][:], rpack[:, h * 16:(h + 1) * 16])
                for j in range(nj):
                    del etiles[(h, j)]

            def emit_patmul(h):
                # broadcast recip_h across 64 partitions via PE outer
                # product into PSUM, then one in-place multiply
                # normalizes the bf16 context.
                t, r = divmod(h, 2)
                rsl = slice(r * 64, (r + 1) * 64)
                for sh in range(2):
                    shsl = slice(sh * 1024, (sh + 1) * 1024)
                    pat_ps = psA2.tile([64, 1024], FP32, tag="psS",
                                       name=f"pat{h}_{sh}")
                    for n in range(2):
                        nsl = slice(sh * 1024 + n * 512,
                                    sh * 1024 + (n + 1) * 512)
                        nc.tensor.matmul(
                            pat_ps[:, n * 512:(n + 1) * 512],
                            ones64_sb[:], recip[h][:, nsl],
                            start=True, stop=True)
                    nc.vector.tensor_mul(ctxb[t][rsl, shsl],
                                         ctxb[t][rsl, shsl], pat_ps[:])

            # software pipeline across heads: head h's scores/exp overlap
            # head h-1's context accumulation; the Q/K t=1 projection
            # groups fill the spare PE capacity of heads 0 and 1
            qt1_at = {j + 1: j for j in range(8)}
            qt1_done = set()
            pat_done = set()
            for h in range(HPC):
                for j in range(nj):
                    emit_scores(h, j)
                    if h == 0 and j in qt1_at:
                        emit_qt1_group(qt1_at[j])
                        qt1_done.add(qt1_at[j])
                    if h > 0:
                        emit_ctx_step(h - 1, j)
                    if j == 2 and h >= 2:
                        emit_patmul(h - 2)
                        pat_done.add(h - 2)
                if h == 0:
                    for n in range(8):   # leftovers when nj is small
                        if n not in qt1_done:
                            emit_qt1_group(n)
                if h > 0 and h < HPC - 1:
                    emit_evict(h - 1)
            for j in range(nj):
                emit_ctx_step(HPC - 1, j)
            emit_evict(HPC - 2)
            emit_evict(HPC - 1)
            for h in range(HPC):
                if h not in pat_done:
                    emit_patmul(h)
            for t in range(NT):
                nc.vector.tensor_scalar_add(
                    ctxb[t][:], ctxb[t][:], bv_sb[:, t:t + 1])

            # ---------------- output projection ----------------
            ostage = st2.enter_context(tc.tile_pool(name="ostage", bufs=3))
            for m in range(S // 128):
                msl = slice(m * 128, (m + 1) * 128)
                if m % 2 == 0:
                    ps = psA2.tile([128, 1024], FP32, tag="psS", name="ps_o")
                else:
                    ps = psC2.tile([128, 1024], FP32, tag="psC", name="ps_o")
                for n2 in range(2):
                    for t in range(NT):
                        nc.tensor.matmul(
                            ps[:, n2 * 512:(n2 + 1) * 512],
                            ctxb[t][:, msl],
                            wo_sb[t][:, n2 * 512:(n2 + 1) * 512],
                            start=(t == 0), stop=(t == NT - 1))
                ob = ostage.tile([128, 1024], BF16, tag="ob", name="ob")
                nc.vector.tensor_copy(ob[:, 0:512], ps[:, 0:512])
                nc.sync.dma_start(out[msl, 0:512], ob[:, 0:512])
                nc.scalar.copy(ob[:, 512:1024], ps[:, 512:1024])
                nc.sync.dma_start(out[msl, 512:1024], ob[:, 512:1024])

    nc.compile()
    return nc


_CACHE = {}


def _graph(nj):
    if nj not in _CACHE:
        _CACHE[nj] = _build(nj)
    return _CACHE[nj]


def _prep_inputs(x, mems, mask, Wq, bq, Wk, bk, Wv, bv, Wo, bo):
    """Shard + preprocess on host. Returns (in_maps, nj)."""
    c = np.concatenate([mems, x], axis=1)          # [B, SKV_FULL, D]
    keep = [np.nonzero(mask[b] != 0)[0] for b in range(B)]
    n_eff = [len(k) for k in keep]
    nj = max(1, (max(n_eff) + 127) // 128)
    skv = nj * 128

    per_batch = []
    for b in range(B):
        ne = n_eff[b]
        cTb = np.zeros((D, skv), ml_dtypes.bfloat16)
        cTb[:, :ne] = c[b][keep[b]].T.astype(ml_dtypes.bfloat16)
        xTb = np.ascontiguousarray(x[b].T.astype(ml_dtypes.bfloat16))
        mbb = np.full(skv, NEG, np.float32)
        mbb[:ne] = 0.0
        mbb = np.ascontiguousarray(mbb.reshape(nj, 128).T)   # [128, nj]
        per_batch.append((xTb, cTb, mbb))

    def fmaj(v):   # [F] -> [128, NT] feature-major
        return np.ascontiguousarray(v.reshape(NT, 128).T.astype(np.float32))

    in_maps = []
    for core in range(N_CORES):
        b, hb = divmod(core, HPC)
        fs = slice(hb * F, (hb + 1) * F)
        xTb, cTb, mbb = per_batch[b]
        in_maps.append({
            "xT": xTb,
            "cT": cTb,
            "wqT": np.ascontiguousarray(Wq[fs, :].T.astype(ml_dtypes.bfloat16)),
            "wkT": np.ascontiguousarray(Wk[fs, :].T.astype(ml_dtypes.bfloat16)),
            "wvT": np.ascontiguousarray(Wv[fs, :].T.astype(ml_dtypes.bfloat16)),
            "woT": np.ascontiguousarray(Wo[:, fs].T.astype(ml_dtypes.bfloat16)),
            "mb": mbb,
            "ones64": np.ones((1, 64), ml_dtypes.bfloat16),
            "bq2": fmaj(bq[fs]),
            "bk2": fmaj(bk[fs]),
            "bv2": fmaj(bv[fs]),
        })
    return in_maps, nj


def _register_ntff_hook():
    try:
        from antenv.axon_hooks import (get_axon_ntff_profile_hook,
                                       set_axon_ntff_profile_hook)
    except ImportError:
        import types

        import antenv
        m = types.ModuleType("antenv.axon_hooks")
        m._hook = None
        m.set_axon_ntff_profile_hook = lambda h: setattr(m, "_hook", h)
        m.get_axon_ntff_profile_hook = lambda: m._hook
        sys.modules["antenv.axon_hooks"] = m
        antenv.axon_hooks = m
        get_axon_ntff_profile_hook = m.get_axon_ntff_profile_hook
        set_axon_ntff_profile_hook = m.set_axon_ntff_profile_hook
    if get_axon_ntff_profile_hook() is None:
        from trn_agent_boot.trn_boot import _ntff_profile_via_ctypes
        set_axon_ntff_profile_hook(
            _ntff_profile_via_ctypes("/opt/axon/libaxon_pjrt.so"))


def _run(inputs, trace=False, trace_kwargs=None):
    x = np.asarray(inputs["x"], np.float32)
    mems = np.asarray(inputs["mems"], np.float32)
    mask = np.asarray(inputs["mask"])
    Wq = np.asarray(inputs["Wq"], np.float32)
    bq = np.asarray(inputs["bq"], np.float32)
    Wk = np.asarray(inputs["Wk"], np.float32)
    bk = np.asarray(inputs["bk"], np.float32)
    Wv = np.asarray(inputs["Wv"], np.float32)
    bv = np.asarray(inputs["bv"], np.float32)
    Wo = np.asarray(inputs["Wo"], np.float32)
    bo = np.asarray(inputs["bo"], np.float32)

    in_maps, nj = _prep_inputs(x, mems, mask, Wq, bq, Wk, bk, Wv, bv, Wo, bo)
    nc = _graph(nj)

    if trace:
        _register_ntff_hook()

    res = run_bass_kernel_spmd(nc, in_maps, core_ids=list(range(N_CORES)),
                               trace=trace, **(trace_kwargs or {}))

    out = np.empty((B, S, D), np.float32)
    for b in range(B):
        acc = res.results[b * HPC]["out"].astype(np.float32)
        for hb in range(1, HPC):
            acc = acc + res.results[b * HPC + hb]["out"].astype(np.float32)
        out[b] = acc + bo[None, :]
    return out, res


def kernel(**inputs) -> np.ndarray:
    out, _ = _run(inputs, trace=False)
    return out

